# revision 22
# baseline (speedup 1.0000x reference)
"""Paged-attention block (QKV proj + QK-RMSNorm + partial RoPE + paged KV attention
+ o_proj) on 8 trn2 NeuronCores, tensor-parallel over heads.

Sharding: core c owns q-heads 4c..4c+3 and kv-head c (shard qkv_weight rows /
o_proj_weight columns / kv caches by head). Each core computes a partial
o_proj output (bf16); the host sums the 8 partials in fp32 (the "allreduce").

All matmul operands are bf16 (same 1 row/cycle PE rate as fp32r, half the
DMA/SBUF footprint, fast weight loads); PSUM accumulation stays fp32.

Schedule: phases are software-pipelined at two levels.
 - qkv(b+1) is emitted before attention(b), so the serial norm+rope+transpose
   latency of b's last token tile hides under the next sequence's projection
   matmuls instead of stalling the PE at each sequence boundary.
 - within attention, the scores matmul runs 3 steps ahead of the PV/
   denominator matmuls so the PE never waits on the ACT-engine exp.
 - o_proj(b) (from SBUF-resident wo) follows attention(b), spreading the
   output DMA across the whole second half of the kernel.
 - only the first qkv-weight chunk is DMA'd before the first hidden tile;
   the rest of wqkv and all of wo queue behind it, so the PE starts ~5us in.
"""

import numpy as np

# problem constants (hardcoded per task contract)
B, SQ, HID = 4, 512, 4096
T = B * SQ
HQ, HKV, D, R = 32, 8, 128, 64
PAGE, MAX_PAGES = 64, 16
CACHED = 512
KV_LEN = CACHED + SQ          # 1024 logical kv positions per sequence
NCORES = 8
GH = HQ // NCORES             # 4 q heads per core
KB = KV_LEN // 128            # 8 kv tiles of 128
NKB = SQ // 128               # 4 new kv tiles
EPS = 1e-6
SCALE = 1.0 / float(D) ** 0.5
NEG = -1.0e30

_COMPILED = None


def _build(reps=1):
    import concourse.tile as tile
    from concourse import mybir, bacc
    from concourse.bass import ds, ts
    from contextlib import ExitStack

    # Every ACT function this kernel uses (Exp, Ln, Square, Copy) lives in
    # the 'natural_log_exp_and_others' piecewise-poly table set, but the
    # greedy per-instruction set assignment alternates between sets, firing
    # a 1.3us LoadActFuncSet on every qkv<->attention phase switch. Restrict
    # the table map (preserving positions, which are the runtime set ids) so
    # one set serves the whole kernel and the load happens exactly once.
    import concourse.hw_specs as hw_specs
    if not hasattr(bacc, "_orig_get_activation_tables"):
        bacc._orig_get_activation_tables = hw_specs.get_activation_tables

        def _single_set_tables(arch):
            tabs = bacc._orig_get_activation_tables(arch)
            keep = "natural_log_exp_and_others"
            if keep not in tabs:
                return tabs
            return {name: (s if name == keep else set())
                    for name, s in tabs.items()}

        bacc.get_activation_tables = _single_set_tables

    bf = mybir.dt.bfloat16
    f32 = mybir.dt.float32

    nc = bacc.Bacc("TRN2", target_bir_lowering=False, debug=False,
                   num_devices=NCORES)

    # hidden, host-pretiled: hTb[m, p, k*128+t] = hidden[m*128+t, k*128+p]
    hT = nc.dram_tensor("hT", (T // 128, 128, HID), bf, kind="ExternalInput")
    wqkv = nc.dram_tensor("wqkv", (HID, (GH + 2) * D), bf, kind="ExternalInput")
    wo = nc.dram_tensor("wo", (GH * D, HID), bf, kind="ExternalInput")
    kcT = nc.dram_tensor("kcT", (B, D, CACHED), bf, kind="ExternalInput")
    # vc host-prepacked: vc[b, p, blk*128+d] = V_cached[b, blk*128+p, d]
    vc = nc.dram_tensor("vc", (B, 128, NKB * 128), bf, kind="ExternalInput")
    cosel = nc.dram_tensor("cosel", (T, R // 2), bf, kind="ExternalInput")
    sinel = nc.dram_tensor("sinel", (T, R // 2), bf, kind="ExternalInput")
    trimask = nc.dram_tensor("trimask", (128, 128), f32, kind="ExternalInput")
    mask3 = nc.dram_tensor("mask3", (128, 256), f32, kind="ExternalInput")
    ident = nc.dram_tensor("ident", (128, 128), bf, kind="ExternalInput")
    onesd = nc.dram_tensor("onesd", (128, 128), bf, kind="ExternalInput")
    outp = nc.dram_tensor("outp", (T, HID), bf, kind="ExternalOutput")

    NF = (GH + 2) * D          # 768 qkv features per core
    NH = GH + 1                # 5 normed+roped heads (4 q + 1 k)

    with tile.TileContext(nc) as tc, ExitStack() as ctx:
        persist = ctx.enter_context(tc.tile_pool(name="persist", bufs=1))
        qt_pool = ctx.enter_context(tc.tile_pool(name="qt", bufs=2))
        kt_pool = ctx.enter_context(tc.tile_pool(name="kt", bufs=2))
        at_pool = ctx.enter_context(tc.tile_pool(name="at", bufs=2))
        work = ctx.enter_context(tc.tile_pool(name="work", bufs=2))
        scratch = ctx.enter_context(tc.tile_pool(name="scratch", bufs=1))
        outpool = ctx.enter_context(tc.tile_pool(name="outstage", bufs=3))
        ps = ctx.enter_context(tc.tile_pool(name="ps", bufs=8, space="PSUM"))

        ident_sb = persist.tile([128, 128], bf, tag="ident")
        tri_sb = persist.tile([128, 128], f32, tag="tri")
        m3_sb = persist.tile([128, 256], f32, tag="m3")
        ones_sb = persist.tile([128, 128], bf, tag="ones")
        eps_sb = persist.tile([128, 1], f32, tag="eps")
        nc.vector.memset(eps_sb[:], EPS)

        half = R // 2  # 32
        mult = mybir.AluOpType.mult
        hT_ap = hT[:].rearrange("m p (ko t) -> m p ko t", t=128)
        wq_ap = wqkv[:].rearrange("(ko p) f -> p ko f", p=128)
        wo_ap = wo[:].rearrange("(ko p) f -> p ko f", p=128)
        cos_ap = cosel[:].rearrange("(bb ml p) h -> bb p ml h", ml=NKB, p=128)
        sin_ap = sinel[:].rearrange("(bb ml p) h -> bb p ml h", ml=NKB, p=128)
        # outp viewed as [b, n, p, ml, f] so one DMA covers a (b, n) block
        out_ap = outp[:].rearrange("(bb ml p) (n f) -> bb n p ml f",
                                   ml=NKB, p=128, f=512)

        for _rep in range(reps):
            with ExitStack() as rctx:
                qph = rctx.enter_context(tc.tile_pool(name="qkvph", bufs=1))
                hpool = rctx.enter_context(tc.tile_pool(name="hstream", bufs=3))
                # resident weights: qkv [128, 32(k), 768] and o_proj
                # [128, 4(ko), 4096]
                wq_sb = qph.tile([128, HID // 128, NF], bf, tag="wq")
                wo_sb = qph.tile([128, GH, HID], bf, tag="wo")
                # first chunk ahead of the hidden stream; the rest defer
                nc.sync.dma_start(wq_sb[:, ts(0, 4), :], wq_ap[:, ts(0, 4), :])

                def emit_qkv(b, first=False, pending_prev=None):
                    """QKV projection + norm + rope + transposes for seq b.

                    The last tile's transposes are NOT emitted here — they're
                    returned as a thunk the caller schedules later, so the
                    PE's in-order queue doesn't stall on the last tile's
                    DVE/ACT norm+rope chain at the phase boundary.
                    Returns ((QT_b, KT_b, V_b), pending_thunk).
                    """
                    QT_b = qt_pool.tile([128, GH, SQ], bf, tag="QT")
                    KT_b = kt_pool.tile([128, SQ], bf, tag="KT")
                    V_b = kt_pool.tile([128, NKB, 128], bf, tag="Vnew")
                    cos_b = work.tile([128, NKB, half], bf, tag="cos", bufs=2)
                    sin_b = work.tile([128, NKB, half], bf, tag="sin", bufs=2)

                    def emit_transposes(ml, qkv_sb):
                        pq = ps.tile([128, 512], bf, tag="ps", name="pq")
                        pk = ps.tile([128, 128], bf, tag="ps", name="pk")
                        for h in range(GH):
                            nc.tensor.transpose(pq[:, ts(h, 128)],
                                                qkv_sb[:, ts(h, D)], ident_sb[:])
                        nc.tensor.transpose(pk[:], qkv_sb[:, ts(GH, D)],
                                            ident_sb[:])
                        nc.vector.tensor_copy(
                            QT_b[:, :, ds(ml * 128, 128)],
                            pq[:].rearrange("p (h t) -> p h t", h=GH))
                        nc.scalar.copy(KT_b[:, ds(ml * 128, 128)], pk[:])

                    def emit_mms(ml, ps_hi, ps_lo, ht_t, ht_t2):
                        for k in range(HID // 128):
                            src = ht_t[:, k, :] if k < 16 else ht_t2[:, k - 16, :]
                            nc.tensor.matmul(ps_hi[:], src, wq_sb[:, k, 0:512],
                                             start=(k == 0), stop=(k == 31))
                            nc.tensor.matmul(ps_lo[:, 0:NF - 512], src,
                                             wq_sb[:, k, 512:NF],
                                             start=(k == 0), stop=(k == 31))

                    def emit_chain(ml, ps_hi, ps_lo):
                        """RMSNorm stats + normalize + rope -> qkv_sb."""
                        x2 = scratch.tile([128, D], f32, tag="x2")
                        ss = work.tile([128, NH], f32, tag="ss")
                        for h5 in range(NH):
                            src_ap = ps_hi[:, ts(h5, D)] if h5 < GH else \
                                ps_lo[:, 0:128]
                            nc.scalar.activation(
                                x2[:], src_ap, mybir.ActivationFunctionType.Square,
                                accum_out=ss[:, ds(h5, 1)])
                        # rstd = (ss/D + eps)^-0.5 via exp(-0.5*ln(.)): Ln and
                        # Exp share one ACT table set with Square/Copy, so the
                        # 1.3us LoadActFuncSet never fires after kernel start
                        # (Sqrt lives in a different set than Exp).
                        rstd = work.tile([128, NH], f32, tag="rstd")
                        nc.scalar.activation(ss[:], ss[:],
                                             mybir.ActivationFunctionType.Ln,
                                             bias=eps_sb[:], scale=1.0 / D)
                        nc.scalar.activation(rstd[:], ss[:],
                                             mybir.ActivationFunctionType.Exp,
                                             scale=-0.5)
                        qkv_sb = work.tile([128, NH * D], bf, tag="qkv_sb", bufs=3)
                        for h5 in range(NH):
                            src_ap = ps_hi[:, ts(h5, D)] if h5 < GH else \
                                ps_lo[:, 0:128]
                            nc.vector.tensor_scalar_mul(
                                qkv_sb[:, ts(h5, D)], src_ap, rstd[:, ds(h5, 1)])
                        nc.scalar.copy(V_b[:, ml, :], ps_lo[:, 128:256])

                        v3 = qkv_sb[:].rearrange("p (h d) -> p h d", h=NH)
                        x1v = v3[:, :, 0:half]
                        x2v = v3[:, :, half:R]
                        cb = cos_b[:, ml, None, :].to_broadcast((128, NH, half))
                        sb_ = sin_b[:, ml, None, :].to_broadcast((128, NH, half))
                        t1 = scratch.tile([128, NH, half], f32, tag="t1")
                        t2 = scratch.tile([128, NH, half], f32, tag="t2")
                        t3 = scratch.tile([128, NH, half], f32, tag="t3")
                        t4 = scratch.tile([128, NH, half], f32, tag="t4")
                        nc.vector.tensor_tensor(t1[:], x1v, cb, mult)
                        nc.vector.tensor_tensor(t2[:], x2v, sb_, mult)
                        nc.vector.tensor_tensor(t3[:], x1v, sb_, mult)
                        nc.vector.tensor_tensor(t4[:], x2v, cb, mult)
                        nc.vector.tensor_tensor(x1v, t1[:], t2[:],
                                                mybir.AluOpType.subtract)
                        nc.vector.tensor_tensor(x2v, t3[:], t4[:],
                                                mybir.AluOpType.add)
                        return qkv_sb

                    ht = {}
                    for ml in range(2 if first else 1):
                        m = b * NKB + ml
                        ht_t = hpool.tile([128, 16, 128], bf, tag="ht", bufs=8)
                        ht_t2 = hpool.tile([128, 16, 128], bf, tag="ht", bufs=8)
                        ht[ml] = (ht_t, ht_t2)
                        nc.sync.dma_start(ht_t[:], hT_ap[m, :, 0:16, :])
                        nc.sync.dma_start(ht_t2[:], hT_ap[m, :, 16:32, :])
                    nc.sync.dma_start(cos_b[:], cos_ap[b])
                    nc.sync.dma_start(sin_b[:], sin_ap[b])
                    if first:
                        # rest of the input stream queues behind tiles 0+1;
                        # later-needed tensors (masks) go last, wo waits for
                        # emit_attention(0)
                        nc.sync.dma_start(ident_sb[:], ident[:])
                        for kq in range(1, 8):
                            nc.sync.dma_start(wq_sb[:, ts(kq, 4), :],
                                              wq_ap[:, ts(kq, 4), :])
                        nc.sync.dma_start(tri_sb[:], trimask[:])
                        nc.sync.dma_start(m3_sb[:], mask3[:])
                        nc.sync.dma_start(ones_sb[:], onesd[:])

                    pend_q = []
                    if first:
                        # Cold start: the wqkv chunk stream (~2.8us/chunk) is
                        # slower than one tile's per-chunk matmul consumption
                        # (~1.3us), so run tiles 0+1 chunk-major (consumption
                        # ~2.6us/chunk) to hide the weight DMA.
                        pp = [(ps.tile([128, 512], f32, tag="ps", name="hi"),
                               ps.tile([128, 512], f32, tag="ps", name="lo"))
                              for _ in range(2)]
                        for k in range(HID // 128):
                            for mlp in range(2):
                                ht_t, ht_t2 = ht[mlp]
                                src = ht_t[:, k, :] if k < 16 else \
                                    ht_t2[:, k - 16, :]
                                nc.tensor.matmul(pp[mlp][0][:], src,
                                                 wq_sb[:, k, 0:512],
                                                 start=(k == 0), stop=(k == 31))
                                nc.tensor.matmul(pp[mlp][1][:, 0:NF - 512], src,
                                                 wq_sb[:, k, 512:NF],
                                                 start=(k == 0), stop=(k == 31))
                        for mlp in range(2):
                            pend_q.append((mlp, emit_chain(mlp, *pp[mlp])))
                        start_ml = 2
                    else:
                        start_ml = 0

                    for ml in range(start_ml, NKB):
                        if ml not in ht:
                            m = b * NKB + ml
                            ht_t = hpool.tile([128, 16, 128], bf, tag="ht",
                                              bufs=8)
                            ht_t2 = hpool.tile([128, 16, 128], bf, tag="ht",
                                               bufs=8)
                            ht[ml] = (ht_t, ht_t2)
                            nc.sync.dma_start(ht_t[:], hT_ap[m, :, 0:16, :])
                            nc.sync.dma_start(ht_t2[:], hT_ap[m, :, 16:32, :])
                        ps_hi = ps.tile([128, 512], f32, tag="ps")
                        ps_lo = ps.tile([128, 512], f32, tag="ps")
                        emit_mms(ml, ps_hi, ps_lo, *ht[ml])
                        if ml == start_ml and pending_prev is not None:
                            pending_prev()
                        if pend_q:
                            emit_transposes(*pend_q.pop(0))
                        pend_q.append((ml, emit_chain(ml, ps_hi, ps_lo)))

                    while len(pend_q) > 1:
                        emit_transposes(*pend_q.pop(0))
                    last = pend_q.pop()
                    pending = lambda: emit_transposes(last[0], last[1])
                    return (QT_b, KT_b, V_b), pending

                def emit_attention(b, tiles, pending_next=None, first=False):
                    """Software-pipelined paged attention for seq b -> aT."""
                    QT_b, KT_b, V_b = tiles
                    kcT_b = kt_pool.tile([128, CACHED], bf, tag="kcT")
                    nc.sync.dma_start(kcT_b[:], kcT[b].rearrange("p k -> p k"))
                    vc_b = kt_pool.tile([128, NKB, 128], bf, tag="vc")
                    nc.sync.dma_start(
                        vc_b[:], vc[b].rearrange("p (blk d) -> p blk d", d=128))
                    if first:
                        for kq in range(8):
                            nc.sync.dma_start(wo_sb[:, :, ds(kq * 512, 512)],
                                              wo_ap[:, :, ds(kq * 512, 512)])

                    aT = at_pool.tile([128, GH, SQ], bf, tag="attnT")
                    steps = [(h, t) for h in range(GH) for t in range(KB)]
                    NS = len(steps)            # 32
                    LOOK = 3
                    e_tiles = [None] * NS
                    offs = [0 if t <= 4 else min((t - 4) * 128, 256)
                            for h, t in steps]
                    acc = {}                   # h -> (outT_ps, den_ps)

                    def stage_A(s):
                        h, t = steps[s]
                        off = offs[s]
                        N = SQ - off
                        lhsT = kcT_b[:, ts(t, 128)] if t < 4 else \
                            KT_b[:, ts(t - 4, 128)]
                        sc_ps = ps.tile([128, 512], f32, tag="ps", name="sc")
                        nc.tensor.matmul(sc_ps[:, off:SQ], lhsT,
                                         QT_b[:, h, off:SQ],
                                         start=True, stop=True)
                        if t == KB - 1:
                            nc.vector.tensor_tensor(sc_ps[:, 256:512],
                                                    sc_ps[:, 256:512],
                                                    m3_sb[:], mybir.AluOpType.add)
                        elif t >= 4:
                            nc.vector.tensor_tensor(
                                sc_ps[:, ds((t - 4) * 128, 128)],
                                sc_ps[:, ds((t - 4) * 128, 128)],
                                tri_sb[:], mybir.AluOpType.add)
                        e_t = work.tile([128, 512], bf, tag="e", bufs=5)
                        nc.scalar.activation(e_t[:, 0:N], sc_ps[:, off:SQ],
                                             mybir.ActivationFunctionType.Exp,
                                             scale=SCALE)
                        e_tiles[s] = e_t

                    def stage_C(s):
                        h, t = steps[s]
                        off = offs[s]
                        N = SQ - off
                        if t == 0:
                            outT_ps = ps.tile([128, 512], f32, tag="ps", name="o")
                            den_ps = ps.tile([128, 512], f32, tag="ps", name="d")
                            acc[h] = (outT_ps, den_ps)
                        outT_ps, den_ps = acc[h]
                        vt = vc_b[:, t, :] if t < 4 else V_b[:, t - 4, :]
                        e_t = e_tiles[s]
                        e_tiles[s] = None
                        nc.tensor.matmul(outT_ps[:, off:SQ], vt, e_t[:, 0:N],
                                         start=(t == 0), stop=(t == KB - 1),
                                         skip_group_check=True)
                        nc.tensor.matmul(den_ps[:, off:SQ], ones_sb[:],
                                         e_t[:, 0:N],
                                         start=(t == 0), stop=(t == KB - 1),
                                         skip_group_check=True)
                        if t == KB - 1:
                            recip = scratch.tile([128, 512], f32, tag="recip",
                                                 bufs=2)
                            nc.vector.reciprocal(recip[:], den_ps[:])
                            nc.vector.tensor_tensor(aT[:, h, :], outT_ps[:],
                                                    recip[:], mult)

                    for s in range(LOOK):
                        stage_A(s)
                    for s in range(NS):
                        stage_C(s)
                        if s == KB - 1 and pending_next is not None:
                            # next seq's deferred last-tile transposes: its
                            # norm+rope chain has had a full head's worth of
                            # attention to finish, so these don't stall PE
                            pending_next()
                        if s + LOOK < NS:
                            stage_A(s + LOOK)
                    return aT

                def emit_oproj(b, aT):
                    """partial_out[b] = aT.T @ woT from resident wo.
                    One output DMA per (b, n) block of [4x128 tok, 512 feat]."""
                    for n in range(HID // 512):
                        ob = outpool.tile([128, NKB, 512], bf, tag="ob")
                        for ml in range(NKB):
                            po = ps.tile([128, 512], f32, tag="ps")
                            for h in range(GH):
                                nc.tensor.matmul(po[:], aT[:, h, ts(ml, 128)],
                                                 wo_sb[:, h, ds(n * 512, 512)],
                                                 start=(h == 0), stop=(h == GH - 1))
                            if (n + ml) % 2 == 0:
                                nc.vector.tensor_copy(ob[:, ml, :], po[:])
                            else:
                                nc.scalar.copy(ob[:, ml, :], po[:])
                        nc.sync.dma_start(out_ap[b, n], ob[:])

                tiles = {}
                pend = {}
                tiles[0], pend[0] = emit_qkv(0, first=True)
                for b in range(B):
                    if b + 1 < B:
                        tiles[b + 1], pend[b + 1] = emit_qkv(
                            b + 1, pending_prev=pend.pop(b, None))
                    aT = emit_attention(b, tiles.pop(b),
                                        pending_next=pend.pop(b + 1, None),
                                        first=(b == 0))
                    emit_oproj(b, aT)

    nc.compile()
    return nc


def _get_compiled():
    global _COMPILED
    if _COMPILED is None:
        _COMPILED = _build()
    return _COMPILED


def _prep_inputs(hidden_states, cos, sin, positions, k_cache, v_cache, page_table,
                 cache_seqlens, cu_seqlens_q, qkv_weight, o_proj_weight,
                 q_norm_weight, k_norm_weight):
    import ml_dtypes
    bf16 = ml_dtypes.bfloat16
    f32 = np.float32
    pos = np.asarray(positions).reshape(B, SQ)
    assert np.array_equal(np.asarray(cache_seqlens),
                          np.full(B, CACHED, np.int32)), "cache_seqlens != CACHED"
    assert np.array_equal(np.asarray(cu_seqlens_q),
                          np.arange(B + 1, dtype=np.int64) * SQ), "cu_seqlens ragged"
    assert (pos == CACHED + np.arange(SQ)[None, :]).all(), "positions ragged"
    assert np.allclose(q_norm_weight, 1.0) and np.allclose(k_norm_weight, 1.0), \
        "non-unit norm weights unsupported"

    pt = np.asarray(page_table)
    phys = (pt[:, :, None] * PAGE + np.arange(PAGE)[None, None, :]).reshape(B, -1)
    slots = pt[np.arange(B)[:, None], pos // PAGE] * PAGE + pos % PAGE
    assert np.array_equal(slots, phys[:, CACHED:]), "non-append page layout"

    kf = np.asarray(k_cache).reshape(-1, HKV, D)
    vf = np.asarray(v_cache).reshape(-1, HKV, D)
    Kc = kf[phys[:, :CACHED]]          # [B, 512, HKV, D]
    Vc = vf[phys[:, :CACHED]]

    cos_sel = np.ascontiguousarray(np.asarray(cos)[positions]).astype(bf16)
    sin_sel = np.ascontiguousarray(np.asarray(sin)[positions]).astype(bf16)
    # hTb[m, p, k*128+t] = hidden[m*128+t, k*128+p]
    hT = np.ascontiguousarray(
        np.asarray(hidden_states, dtype=f32).reshape(T // 128, 128, HID // 128, 128)
        .transpose(0, 3, 2, 1).reshape(T // 128, 128, HID)).astype(bf16)
    tri = np.where(np.arange(128)[None, :] >= np.arange(128)[:, None],
                   np.float32(0.0), np.float32(NEG))
    m3 = np.concatenate([np.full((128, 128), NEG, f32), tri], axis=1)
    eye = np.eye(128, dtype=bf16)

    qw = np.asarray(qkv_weight)
    ow = np.asarray(o_proj_weight)
    in_maps = []
    for c in range(NCORES):
        rows = np.concatenate([
            qw[c * GH * D:(c + 1) * GH * D],
            qw[HQ * D + c * D: HQ * D + (c + 1) * D],
            qw[HQ * D + HKV * D + c * D: HQ * D + HKV * D + (c + 1) * D],
        ], axis=0)
        in_maps.append(dict(
            hT=hT,
            wqkv=np.ascontiguousarray(rows.T).astype(bf16),
            wo=np.ascontiguousarray(ow[:, c * GH * D:(c + 1) * GH * D].T).astype(bf16),
            kcT=np.ascontiguousarray(Kc[:, :, c, :].transpose(0, 2, 1)).astype(bf16),
            # vc prepacked so each partition row p holds [blk, d] contiguously:
            # vc[b, p, blk*128+d] = Vc[b, blk*128+p, d]
            vc=np.ascontiguousarray(
                Vc[:, :, c, :].reshape(B, NKB, 128, D).transpose(0, 2, 1, 3)
                .reshape(B, 128, NKB * D)).astype(bf16),
            cosel=cos_sel, sinel=sin_sel, trimask=tri, mask3=m3,
            ident=eye, onesd=np.ones((128, 128), dtype=bf16),
        ))
    return in_maps


def kernel(**inputs) -> np.ndarray:
    from concourse.bass_utils import run_bass_kernel_spmd
    in_maps = _prep_inputs(**inputs)
    nc = _get_compiled()
    res = run_bass_kernel_spmd(nc, in_maps, core_ids=list(range(NCORES)))
    acc = res.results[0]["outp"].astype(np.float32)
    for c in range(1, NCORES):
        acc += res.results[c]["outp"].astype(np.float32)
    return acc


# revision 28
# speedup vs baseline: 1.3251x; 1.3251x over previous
"""Paged-attention block (QKV proj + QK-RMSNorm + partial RoPE + paged KV attention
+ o_proj) on 8 trn2 NeuronCores, tensor-parallel over heads.

Sharding: core c owns q-heads 4c..4c+3 and kv-head c (shard qkv_weight rows /
o_proj_weight columns / kv caches by head). Each core computes a partial
o_proj output (bf16); the host sums the 8 partials in fp32 (the "allreduce").

All matmul operands are bf16 (same 1 row/cycle PE rate as fp32r, half the
DMA/SBUF footprint, fast weight loads); PSUM accumulation stays fp32.

Schedule: phases are software-pipelined at two levels.
 - qkv(b+1) is emitted before attention(b), so the serial norm+rope+transpose
   latency of b's last token tile hides under the next sequence's projection
   matmuls instead of stalling the PE at each sequence boundary.
 - within attention, the scores matmul runs 3 steps ahead of the PV/
   denominator matmuls so the PE never waits on the ACT-engine exp.
 - o_proj(b) (from SBUF-resident wo) follows attention(b), spreading the
   output DMA across the whole second half of the kernel.
 - only the first qkv-weight chunk is DMA'd before the first hidden tile;
   the rest of wqkv and all of wo queue behind it, so the PE starts ~5us in.
"""

import numpy as np

# problem constants (hardcoded per task contract)
B, SQ, HID = 4, 512, 4096
T = B * SQ
HQ, HKV, D, R = 32, 8, 128, 64
PAGE, MAX_PAGES = 64, 16
CACHED = 512
KV_LEN = CACHED + SQ          # 1024 logical kv positions per sequence
NCORES = 8
GH = HQ // NCORES             # 4 q heads per core
KB = KV_LEN // 128            # 8 kv tiles of 128
NKB = SQ // 128               # 4 new kv tiles
EPS = 1e-6
SCALE = 1.0 / float(D) ** 0.5
NEG = -1.0e30

_COMPILED = None


def _build(reps=1):
    import concourse.tile as tile
    from concourse import mybir, bacc
    from concourse.bass import ds, ts
    from contextlib import ExitStack

    # Every ACT function this kernel uses (Exp, Ln, Square, Copy) lives in
    # the 'natural_log_exp_and_others' piecewise-poly table set, but the
    # greedy per-instruction set assignment alternates between sets, firing
    # a 1.3us LoadActFuncSet on every qkv<->attention phase switch. Restrict
    # the table map (preserving positions, which are the runtime set ids) so
    # one set serves the whole kernel and the load happens exactly once.
    import concourse.hw_specs as hw_specs
    if not hasattr(bacc, "_orig_get_activation_tables"):
        bacc._orig_get_activation_tables = hw_specs.get_activation_tables

        def _single_set_tables(arch):
            tabs = bacc._orig_get_activation_tables(arch)
            keep = "natural_log_exp_and_others"
            if keep not in tabs:
                return tabs
            return {name: (s if name == keep else set())
                    for name, s in tabs.items()}

        bacc.get_activation_tables = _single_set_tables

    bf = mybir.dt.bfloat16
    f32 = mybir.dt.float32

    nc = bacc.Bacc("TRN2", target_bir_lowering=False, debug=False,
                   num_devices=NCORES)

    # hidden, host-pretiled: hTb[m, p, k*128+t] = hidden[m*128+t, k*128+p]
    hT = nc.dram_tensor("hT", (T // 128, 128, HID), bf, kind="ExternalInput")
    wqkv = nc.dram_tensor("wqkv", (HID, (GH + 2) * D), bf, kind="ExternalInput")
    wo = nc.dram_tensor("wo", (GH * D, HID), bf, kind="ExternalInput")
    kcT = nc.dram_tensor("kcT", (B, D, CACHED), bf, kind="ExternalInput")
    # vc host-prepacked: vc[b, p, blk*128+d] = V_cached[b, blk*128+p, d]
    vc = nc.dram_tensor("vc", (B, 128, NKB * 128), bf, kind="ExternalInput")
    cosel = nc.dram_tensor("cosel", (T, R // 2), bf, kind="ExternalInput")
    sinel = nc.dram_tensor("sinel", (T, R // 2), bf, kind="ExternalInput")
    trimask = nc.dram_tensor("trimask", (128, 128), bf, kind="ExternalInput")
    mask3 = nc.dram_tensor("mask3", (128, 256), bf, kind="ExternalInput")
    ident = nc.dram_tensor("ident", (128, 128), bf, kind="ExternalInput")
    onesd = nc.dram_tensor("onesd", (128, 128), bf, kind="ExternalInput")
    outp = nc.dram_tensor("outp", (T, HID), bf, kind="ExternalOutput")

    NF = (GH + 2) * D          # 768 qkv features per core
    NH = GH + 1                # 5 normed+roped heads (4 q + 1 k)

    with tile.TileContext(nc) as tc, ExitStack() as ctx:
        persist = ctx.enter_context(tc.tile_pool(name="persist", bufs=1))
        qt_pool = ctx.enter_context(tc.tile_pool(name="qt", bufs=2))
        kt_pool = ctx.enter_context(tc.tile_pool(name="kt", bufs=2))
        at_pool = ctx.enter_context(tc.tile_pool(name="at", bufs=2))
        work = ctx.enter_context(tc.tile_pool(name="work", bufs=2))
        scratch = ctx.enter_context(tc.tile_pool(name="scratch", bufs=1))
        outpool = ctx.enter_context(tc.tile_pool(name="outstage", bufs=3))
        ps = ctx.enter_context(tc.tile_pool(name="ps", bufs=8, space="PSUM"))

        ident_sb = persist.tile([128, 128], bf, tag="ident")
        tri_sb = persist.tile([128, 128], bf, tag="tri")
        m3_sb = persist.tile([128, 256], bf, tag="m3")
        ones_sb = persist.tile([128, 128], bf, tag="ones")
        eps_sb = persist.tile([128, 1], f32, tag="eps")
        nc.vector.memset(eps_sb[:], EPS)

        half = R // 2  # 32
        mult = mybir.AluOpType.mult
        hT_ap = hT[:].rearrange("m p (ko t) -> m p ko t", t=128)
        wq_ap = wqkv[:].rearrange("(ko p) f -> p ko f", p=128)
        wo_ap = wo[:].rearrange("(ko p) f -> p ko f", p=128)
        cos_ap = cosel[:].rearrange("(bb ml p) h -> bb p ml h", ml=NKB, p=128)
        sin_ap = sinel[:].rearrange("(bb ml p) h -> bb p ml h", ml=NKB, p=128)
        # outp viewed as [b, n, p, ml, f] so one DMA covers a (b, n) block
        out_ap = outp[:].rearrange("(bb ml p) (n f) -> bb n p ml f",
                                   ml=NKB, p=128, f=512)

        for _rep in range(reps):
            with ExitStack() as rctx:
                qph = rctx.enter_context(tc.tile_pool(name="qkvph", bufs=1))
                hpool = rctx.enter_context(tc.tile_pool(name="hstream", bufs=3))
                # resident weights: qkv [128, 32(k), 768] and o_proj
                # [128, 4(ko), 4096]
                wq_sb = qph.tile([128, HID // 128, NF], bf, tag="wq")
                wo_sb = qph.tile([128, GH, HID], bf, tag="wo")
                # first chunk ahead of the hidden stream; the rest defer
                nc.sync.dma_start(wq_sb[:, ts(0, 4), :], wq_ap[:, ts(0, 4), :])

                def emit_qkv(b, first=False, pending_prev=None):
                    """QKV projection + norm + rope + transposes for seq b.

                    The last tile's transposes are NOT emitted here — they're
                    returned as a thunk the caller schedules later, so the
                    PE's in-order queue doesn't stall on the last tile's
                    DVE/ACT norm+rope chain at the phase boundary.
                    Returns ((QT_b, KT_b, V_b), pending_thunk).
                    """
                    QT_b = qt_pool.tile([128, GH, SQ], bf, tag="QT")
                    KT_b = kt_pool.tile([128, SQ], bf, tag="KT")
                    V_b = kt_pool.tile([128, NKB, 128], bf, tag="Vnew")
                    cos_b = work.tile([128, NKB, half], bf, tag="cos", bufs=2)
                    sin_b = work.tile([128, NKB, half], bf, tag="sin", bufs=2)

                    def emit_transposes(ml, qkv_sb):
                        pq = ps.tile([128, 512], bf, tag="ps", name="pq")
                        pk = ps.tile([128, 128], bf, tag="ps", name="pk")
                        for h in range(GH):
                            nc.tensor.transpose(pq[:, ts(h, 128)],
                                                qkv_sb[:, ts(h, D)], ident_sb[:])
                        nc.tensor.transpose(pk[:], qkv_sb[:, ts(GH, D)],
                                            ident_sb[:])
                        nc.vector.tensor_copy(
                            QT_b[:, :, ds(ml * 128, 128)],
                            pq[:].rearrange("p (h t) -> p h t", h=GH))
                        nc.scalar.copy(KT_b[:, ds(ml * 128, 128)], pk[:])

                    def emit_mms(ml, ps_hi, ps_lo, ht_t, ht_t2):
                        for k in range(HID // 128):
                            src = ht_t[:, k, :] if k < 16 else ht_t2[:, k - 16, :]
                            nc.tensor.matmul(ps_hi[:], src, wq_sb[:, k, 0:512],
                                             start=(k == 0), stop=(k == 31))
                            nc.tensor.matmul(ps_lo[:, 0:NF - 512], src,
                                             wq_sb[:, k, 512:NF],
                                             start=(k == 0), stop=(k == 31))

                    def emit_chain(ml, ps_hi, ps_lo):
                        """RMSNorm stats + normalize + rope -> qkv_sb."""
                        x2 = scratch.tile([128, D], f32, tag="x2")
                        ss = work.tile([128, NH], f32, tag="ss")
                        for h5 in range(NH):
                            src_ap = ps_hi[:, ts(h5, D)] if h5 < GH else \
                                ps_lo[:, 0:128]
                            nc.scalar.activation(
                                x2[:], src_ap, mybir.ActivationFunctionType.Square,
                                accum_out=ss[:, ds(h5, 1)])
                        # rstd = (ss/D + eps)^-0.5 via exp(-0.5*ln(.)): Ln and
                        # Exp share one ACT table set with Square/Copy, so the
                        # 1.3us LoadActFuncSet never fires after kernel start
                        # (Sqrt lives in a different set than Exp).
                        rstd = work.tile([128, NH], f32, tag="rstd")
                        nc.scalar.activation(ss[:], ss[:],
                                             mybir.ActivationFunctionType.Ln,
                                             bias=eps_sb[:], scale=1.0 / D)
                        nc.scalar.activation(rstd[:], ss[:],
                                             mybir.ActivationFunctionType.Exp,
                                             scale=-0.5)
                        qkv_sb = work.tile([128, NH * D], bf, tag="qkv_sb", bufs=3)
                        for h5 in range(NH):
                            src_ap = ps_hi[:, ts(h5, D)] if h5 < GH else \
                                ps_lo[:, 0:128]
                            nc.vector.tensor_scalar_mul(
                                qkv_sb[:, ts(h5, D)], src_ap, rstd[:, ds(h5, 1)])
                        nc.scalar.copy(V_b[:, ml, :], ps_lo[:, 128:256])

                        v3 = qkv_sb[:].rearrange("p (h d) -> p h d", h=NH)
                        x1v = v3[:, :, 0:half]
                        x2v = v3[:, :, half:R]
                        cb = cos_b[:, ml, None, :].to_broadcast((128, NH, half))
                        sb_ = sin_b[:, ml, None, :].to_broadcast((128, NH, half))
                        t1 = scratch.tile([128, NH, half], f32, tag="t1")
                        t2 = scratch.tile([128, NH, half], f32, tag="t2")
                        t3 = scratch.tile([128, NH, half], f32, tag="t3")
                        t4 = scratch.tile([128, NH, half], f32, tag="t4")
                        nc.vector.tensor_tensor(t1[:], x1v, cb, mult)
                        nc.vector.tensor_tensor(t2[:], x2v, sb_, mult)
                        nc.vector.tensor_tensor(t3[:], x1v, sb_, mult)
                        nc.vector.tensor_tensor(t4[:], x2v, cb, mult)
                        nc.vector.tensor_tensor(x1v, t1[:], t2[:],
                                                mybir.AluOpType.subtract)
                        nc.vector.tensor_tensor(x2v, t3[:], t4[:],
                                                mybir.AluOpType.add)
                        return qkv_sb

                    ht = {}
                    for ml in range(2 if first else 1):
                        m = b * NKB + ml
                        ht_t = hpool.tile([128, 16, 128], bf, tag="ht", bufs=8)
                        ht_t2 = hpool.tile([128, 16, 128], bf, tag="ht", bufs=8)
                        ht[ml] = (ht_t, ht_t2)
                        nc.sync.dma_start(ht_t[:], hT_ap[m, :, 0:16, :])
                        nc.sync.dma_start(ht_t2[:], hT_ap[m, :, 16:32, :])
                    nc.sync.dma_start(cos_b[:], cos_ap[b])
                    nc.sync.dma_start(sin_b[:], sin_ap[b])
                    if first:
                        # rest of the input stream queues behind tiles 0+1;
                        # later-needed tensors (masks) go last, wo waits for
                        # emit_attention(0)
                        nc.sync.dma_start(ident_sb[:], ident[:])
                        for kq in range(1, 8):
                            nc.sync.dma_start(wq_sb[:, ts(kq, 4), :],
                                              wq_ap[:, ts(kq, 4), :])
                        nc.sync.dma_start(tri_sb[:], trimask[:])
                        nc.sync.dma_start(m3_sb[:], mask3[:])
                        nc.sync.dma_start(ones_sb[:], onesd[:])

                    pend_q = []
                    if first:
                        # Cold start: the wqkv chunk stream (~2.8us/chunk) is
                        # slower than one tile's per-chunk matmul consumption
                        # (~1.3us), so run tiles 0+1 chunk-major (consumption
                        # ~2.6us/chunk) to hide the weight DMA.
                        pp = [(ps.tile([128, 512], f32, tag="ps", name="hi"),
                               ps.tile([128, 512], f32, tag="ps", name="lo"))
                              for _ in range(2)]
                        for k in range(HID // 128):
                            for mlp in range(2):
                                ht_t, ht_t2 = ht[mlp]
                                src = ht_t[:, k, :] if k < 16 else \
                                    ht_t2[:, k - 16, :]
                                nc.tensor.matmul(pp[mlp][0][:], src,
                                                 wq_sb[:, k, 0:512],
                                                 start=(k == 0), stop=(k == 31))
                                nc.tensor.matmul(pp[mlp][1][:, 0:NF - 512], src,
                                                 wq_sb[:, k, 512:NF],
                                                 start=(k == 0), stop=(k == 31))
                        for mlp in range(2):
                            pend_q.append((mlp, emit_chain(mlp, *pp[mlp])))
                        start_ml = 2
                    else:
                        start_ml = 0

                    for ml in range(start_ml, NKB):
                        if ml not in ht:
                            m = b * NKB + ml
                            ht_t = hpool.tile([128, 16, 128], bf, tag="ht",
                                              bufs=8)
                            ht_t2 = hpool.tile([128, 16, 128], bf, tag="ht",
                                               bufs=8)
                            ht[ml] = (ht_t, ht_t2)
                            nc.sync.dma_start(ht_t[:], hT_ap[m, :, 0:16, :])
                            nc.sync.dma_start(ht_t2[:], hT_ap[m, :, 16:32, :])
                        ps_hi = ps.tile([128, 512], f32, tag="ps")
                        ps_lo = ps.tile([128, 512], f32, tag="ps")
                        emit_mms(ml, ps_hi, ps_lo, *ht[ml])
                        if ml == start_ml and pending_prev is not None:
                            pending_prev()
                        if pend_q:
                            emit_transposes(*pend_q.pop(0))
                        pend_q.append((ml, emit_chain(ml, ps_hi, ps_lo)))

                    while len(pend_q) > 1:
                        emit_transposes(*pend_q.pop(0))
                    last = pend_q.pop()
                    pending = lambda: emit_transposes(last[0], last[1])
                    return (QT_b, KT_b, V_b), pending

                def emit_attention(b, tiles, pending_next=None, first=False):
                    """Software-pipelined paged attention for seq b -> aT."""
                    QT_b, KT_b, V_b = tiles
                    kcT_b = kt_pool.tile([128, CACHED], bf, tag="kcT")
                    nc.sync.dma_start(kcT_b[:], kcT[b].rearrange("p k -> p k"))
                    vc_b = kt_pool.tile([128, NKB, 128], bf, tag="vc")
                    nc.sync.dma_start(
                        vc_b[:], vc[b].rearrange("p (blk d) -> p blk d", d=128))
                    if first:
                        for kq in range(8):
                            nc.sync.dma_start(wo_sb[:, :, ds(kq * 512, 512)],
                                              wo_ap[:, :, ds(kq * 512, 512)])

                    aT = at_pool.tile([128, GH, SQ], bf, tag="attnT")
                    steps = [(h, t) for h in range(GH) for t in range(KB)]
                    NS = len(steps)            # 32
                    LOOK = 3
                    e_tiles = [None] * NS
                    offs = [0 if t <= 4 else min((t - 4) * 128, 256)
                            for h, t in steps]
                    acc = {}                   # h -> (outT_ps, den_ps)

                    def stage_A(s):
                        h, t = steps[s]
                        off = offs[s]
                        N = SQ - off
                        lhsT = kcT_b[:, ts(t, 128)] if t < 4 else \
                            KT_b[:, ts(t - 4, 128)]
                        sc_ps = ps.tile([128, 512], f32, tag="ps", name="sc")
                        nc.tensor.matmul(sc_ps[:, off:SQ], lhsT,
                                         QT_b[:, h, off:SQ],
                                         start=True, stop=(t < 4),
                                         skip_group_check=True)
                        # causal mask folded in on the PE (identity @ mask
                        # accumulates into the scores PSUM): keeps the
                        # scores->exp chain free of a DVE hop
                        if t == KB - 1:
                            nc.tensor.matmul(sc_ps[:, 256:512], ident_sb[:],
                                             m3_sb[:], start=False, stop=True,
                                             skip_group_check=True)
                        elif t >= 4:
                            nc.tensor.matmul(sc_ps[:, ds((t - 4) * 128, 128)],
                                             ident_sb[:], tri_sb[:],
                                             start=False, stop=True,
                                             skip_group_check=True)
                        e_t = work.tile([128, 512], bf, tag="e", bufs=5)
                        nc.scalar.activation(e_t[:, 0:N], sc_ps[:, off:SQ],
                                             mybir.ActivationFunctionType.Exp,
                                             scale=SCALE)
                        e_tiles[s] = e_t

                    def stage_C(s):
                        h, t = steps[s]
                        off = offs[s]
                        N = SQ - off
                        if t == 0:
                            outT_ps = ps.tile([128, 512], f32, tag="ps", name="o")
                            den_ps = ps.tile([128, 512], f32, tag="ps", name="d")
                            acc[h] = (outT_ps, den_ps)
                        outT_ps, den_ps = acc[h]
                        vt = vc_b[:, t, :] if t < 4 else V_b[:, t - 4, :]
                        e_t = e_tiles[s]
                        e_tiles[s] = None
                        nc.tensor.matmul(outT_ps[:, off:SQ], vt, e_t[:, 0:N],
                                         start=(t == 0), stop=(t == KB - 1),
                                         skip_group_check=True)
                        nc.tensor.matmul(den_ps[:, off:SQ], ones_sb[:],
                                         e_t[:, 0:N],
                                         start=(t == 0), stop=(t == KB - 1),
                                         skip_group_check=True)
                        if t == KB - 1:
                            recip = scratch.tile([128, 512], f32, tag="recip",
                                                 bufs=2)
                            nc.vector.reciprocal(recip[:], den_ps[:])
                            nc.vector.tensor_tensor(aT[:, h, :], outT_ps[:],
                                                    recip[:], mult)

                    for s in range(LOOK):
                        stage_A(s)
                    for s in range(NS):
                        if s + LOOK < NS:
                            stage_A(s + LOOK)
                        stage_C(s)
                        if s == KB - 1 and pending_next is not None:
                            # next seq's deferred last-tile transposes: its
                            # norm+rope chain has had a full head's worth of
                            # attention to finish, so these don't stall PE
                            pending_next()
                    return aT

                def emit_oproj(b, aT):
                    """partial_out[b] = aT.T @ woT from resident wo.
                    One output DMA per (b, n) block of [4x128 tok, 512 feat]."""
                    for n in range(HID // 512):
                        ob = outpool.tile([128, NKB, 512], bf, tag="ob")
                        for ml in range(NKB):
                            po = ps.tile([128, 512], f32, tag="ps")
                            for h in range(GH):
                                nc.tensor.matmul(po[:], aT[:, h, ts(ml, 128)],
                                                 wo_sb[:, h, ds(n * 512, 512)],
                                                 start=(h == 0), stop=(h == GH - 1))
                            if (n + ml) % 2 == 0:
                                nc.vector.tensor_copy(ob[:, ml, :], po[:])
                            else:
                                nc.scalar.copy(ob[:, ml, :], po[:])
                        nc.sync.dma_start(out_ap[b, n], ob[:])

                tiles = {}
                pend = {}
                tiles[0], pend[0] = emit_qkv(0, first=True)
                for b in range(B):
                    if b + 1 < B:
                        tiles[b + 1], pend[b + 1] = emit_qkv(
                            b + 1, pending_prev=pend.pop(b, None))
                    aT = emit_attention(b, tiles.pop(b),
                                        pending_next=pend.pop(b + 1, None),
                                        first=(b == 0))
                    emit_oproj(b, aT)

    nc.compile()
    return nc


def _get_compiled():
    global _COMPILED
    if _COMPILED is None:
        _COMPILED = _build()
    return _COMPILED


def _prep_inputs(hidden_states, cos, sin, positions, k_cache, v_cache, page_table,
                 cache_seqlens, cu_seqlens_q, qkv_weight, o_proj_weight,
                 q_norm_weight, k_norm_weight):
    import ml_dtypes
    bf16 = ml_dtypes.bfloat16
    f32 = np.float32
    pos = np.asarray(positions).reshape(B, SQ)
    assert np.array_equal(np.asarray(cache_seqlens),
                          np.full(B, CACHED, np.int32)), "cache_seqlens != CACHED"
    assert np.array_equal(np.asarray(cu_seqlens_q),
                          np.arange(B + 1, dtype=np.int64) * SQ), "cu_seqlens ragged"
    assert (pos == CACHED + np.arange(SQ)[None, :]).all(), "positions ragged"
    assert np.allclose(q_norm_weight, 1.0) and np.allclose(k_norm_weight, 1.0), \
        "non-unit norm weights unsupported"

    pt = np.asarray(page_table)
    phys = (pt[:, :, None] * PAGE + np.arange(PAGE)[None, None, :]).reshape(B, -1)
    slots = pt[np.arange(B)[:, None], pos // PAGE] * PAGE + pos % PAGE
    assert np.array_equal(slots, phys[:, CACHED:]), "non-append page layout"

    kf = np.asarray(k_cache).reshape(-1, HKV, D)
    vf = np.asarray(v_cache).reshape(-1, HKV, D)
    Kc = kf[phys[:, :CACHED]]          # [B, 512, HKV, D]
    Vc = vf[phys[:, :CACHED]]

    cos_sel = np.ascontiguousarray(np.asarray(cos)[positions]).astype(bf16)
    sin_sel = np.ascontiguousarray(np.asarray(sin)[positions]).astype(bf16)
    # hTb[m, p, k*128+t] = hidden[m*128+t, k*128+p]
    hT = np.ascontiguousarray(
        np.asarray(hidden_states, dtype=f32).reshape(T // 128, 128, HID // 128, 128)
        .transpose(0, 3, 2, 1).reshape(T // 128, 128, HID)).astype(bf16)
    tri = np.where(np.arange(128)[None, :] >= np.arange(128)[:, None],
                   np.float32(0.0), np.float32(NEG)).astype(bf16)
    m3 = np.concatenate([np.full((128, 128), NEG, f32).astype(bf16), tri],
                        axis=1)
    eye = np.eye(128, dtype=bf16)

    qw = np.asarray(qkv_weight)
    ow = np.asarray(o_proj_weight)
    in_maps = []
    for c in range(NCORES):
        rows = np.concatenate([
            qw[c * GH * D:(c + 1) * GH * D],
            qw[HQ * D + c * D: HQ * D + (c + 1) * D],
            qw[HQ * D + HKV * D + c * D: HQ * D + HKV * D + (c + 1) * D],
        ], axis=0)
        in_maps.append(dict(
            hT=hT,
            wqkv=np.ascontiguousarray(rows.T).astype(bf16),
            wo=np.ascontiguousarray(ow[:, c * GH * D:(c + 1) * GH * D].T).astype(bf16),
            kcT=np.ascontiguousarray(Kc[:, :, c, :].transpose(0, 2, 1)).astype(bf16),
            # vc prepacked so each partition row p holds [blk, d] contiguously:
            # vc[b, p, blk*128+d] = Vc[b, blk*128+p, d]
            vc=np.ascontiguousarray(
                Vc[:, :, c, :].reshape(B, NKB, 128, D).transpose(0, 2, 1, 3)
                .reshape(B, 128, NKB * D)).astype(bf16),
            cosel=cos_sel, sinel=sin_sel, trimask=tri, mask3=m3,
            ident=eye, onesd=np.ones((128, 128), dtype=bf16),
        ))
    return in_maps


def kernel(**inputs) -> np.ndarray:
    from concourse.bass_utils import run_bass_kernel_spmd
    in_maps = _prep_inputs(**inputs)
    nc = _get_compiled()
    res = run_bass_kernel_spmd(nc, in_maps, core_ids=list(range(NCORES)))
    acc = res.results[0]["outp"].astype(np.float32)
    for c in range(1, NCORES):
        acc += res.results[c]["outp"].astype(np.float32)
    return acc
